# revision 12
# baseline (speedup 1.0000x reference)
"""AWQ fused dequant + GEMM, tensor-parallel over 8 Trainium2 NeuronCores.

Problem: out[b,s,n] = sum_k x[b,s,k] * W[n,k] + bias[n]
         W[n,k] = qweight[n,k] * scales[k//gs, n] + scaled_zeros[k//gs, n]
Shapes:  x [4,512,4096] fp16, qweight [11008,4096] int32 (values 0..15),
         scales/scaled_zeros [32,11008] fp16, bias [11008] fp16, gs=128.

Strategy (column-parallel, no collectives):
  - Shard N=11008 across 8 cores (1376 each); x replicated.
  - Host prep per core packs [q | scales' | zeros'] into one
    [KT, 128, 3*NC] fp16 tensor ("dsp"), one DMA per k-tile, so each
    on-chip dequant DVE op has exactly one DMA dependency.
  - GEMM dtype MODE:
      int16: fixed-point GEMM. x and W are scaled by power-of-two
        factors (exact in fp16) and rounded to int16; PE int16 matmul
        runs ~2x faster than fp16 on this silicon while the quantization
        error stays ~1e-4 absolute relative to output scale.
      bf16 : cast x/W to bf16 (~4e-3 rel err).
      fp16 : exact fp16 pipeline (slowest PE rate).
  - On chip per core: dequant W[k] tiles [128, NC] (resident,
    ~86KB/partition), stream x m-tiles, accumulate out[m,:] over KT
    k-matmuls per PSUM chunk (512/512/352), epilogue rescale+bias on
    DVE, store [128, NC] fp16 per m-tile.
"""

import numpy as np
import ml_dtypes

import concourse.bass as bass
import concourse.mybir as mybir
from concourse import bacc
from concourse.tile import TileContext
from concourse.bass_utils import run_bass_kernel_spmd

P = 128          # partitions / PE tile
N_CORES = 8
PSUM_CHUNK = 512

MODE = "mixed"   # "fp16" | "bf16" | "mixed" | "int16"
# "mixed": x (stationary operand) in bf16, W (moving) exact fp16 — the PE
# streams at full rate when the stationary dtype is bf16, ~2x faster than
# an fp16 stationary, while W keeps fp16 precision.
X_SHIFT = 12     # x scale 2^-X_SHIFT   (int16 mode)
W_SHIFT = 15     # W scale 2^-W_SHIFT   (int16 mode)


def _chunks(n, step=PSUM_CHUNK):
    out, c0 = [], 0
    while c0 < n:
        out.append((c0, min(step, n - c0)))
        c0 += step
    return out


def _raw_matmul(nc, out, lhsT, rhs, start, stop):
    """nc.tensor.matmul without the dtype whitelist (for int16 operands)."""
    ifmap_ap = nc.tensor.lower_ap(rhs.opt({0}), opt=False)
    weights_ap = nc.tensor.lower_ap(lhsT.opt({0}), opt=False, for_matmul_weights=True)
    out_ap = nc.tensor.lower_ap(out)
    return nc.tensor.add_instruction(
        mybir.InstMatmult(
            name=nc.get_next_instruction_name(),
            replication_resolution=0,
            replication_shift_amnt=0,
            replication_num_rows=0,
            start_tensor_calc=start,
            stop_tensor_calc=stop,
            ins=[ifmap_ap, weights_ap],
            outs=[out_ap],
            perf_mode=None,
            is_transpose=None,
            ifmap_quant_offset=None,
            weights_quant_offset=None,
            bass_skip_group_check=False,
            tile_position=(0, 0),
            tile_size=(128, 128),
        )
    )


def build_bass(M, K, NC, repeat=1, mode=MODE):
    """Build the per-core SPMD Bass program.

    DRAM parameter layouts (host-prepped):
      xp  [MT, P, K]     xp[mt, k_in, kt*P + m_in] = x[mt*P+m_in, kt*P+k_in]
                         (fp16 / bf16 / int16 per mode)
      dsp [KT, P, 3*NC]  fp16, per k-tile [ qT | scales' bcast | zeros' bcast ]
      bp  [P, NC]        fp16, bias broadcast across partitions
      op  [MT, P, NC]    fp16 output, op[mt, m_in, n]
    """
    MT, KT = M // P, K // P
    f16, f32, bf16, i16 = (
        mybir.dt.float16, mybir.dt.float32, mybir.dt.bfloat16, mybir.dt.int16
    )
    mult, add = mybir.AluOpType.mult, mybir.AluOpType.add
    x_dt = {"fp16": f16, "bf16": bf16, "mixed": bf16, "int16": i16}[mode]
    w_dt = {"fp16": f16, "bf16": bf16, "mixed": f16, "int16": i16}[mode]

    nc = bacc.Bacc(None, target_bir_lowering=False, debug=True)
    x_in = nc.declare_dram_parameter("xp", [MT, P, K], x_dt, isOutput=False)
    d_in = nc.declare_dram_parameter("dsp", [KT, P, 3 * NC], f16, isOutput=False)
    b_in = nc.declare_dram_parameter("bp", [P, NC], f16, isOutput=False)
    o_out = nc.declare_dram_parameter("op", [MT, P, NC], f16, isOutput=True)

    def mm(out, lhsT, rhs, start, stop):
        if mode == "int16":
            _raw_matmul(nc, out, lhsT, rhs, start, stop)
        else:
            nc.tensor.matmul(out, lhsT, rhs, start=start, stop=stop)

    with TileContext(nc) as tc:
        with (
            tc.tile_pool(name="wpool", bufs=KT) as wpool,
            tc.tile_pool(name="xpool", bufs=3) as xpool,
            tc.tile_pool(name="dpool", bufs=3) as dpool,
            tc.tile_pool(name="tpool", bufs=2) as tpool,
            tc.tile_pool(name="cpool", bufs=1) as cpool,
            tc.tile_pool(name="opool", bufs=MT) as opool,
            tc.tile_pool(name="pspool", bufs=6, space="PSUM") as pspool,
            tc.tile_pool(name="epool", bufs=3) as epool,
        ):
            bias_t = cpool.tile([P, NC], f16)
            nc.sync.dma_start(out=bias_t[:], in_=b_in[:])
            # Make DVE observe the bias DMA's semaphore lane early so the
            # epilogue ops don't need an extra sync-wait slot for it.
            scratch = cpool.tile([P, 1], f16)
            nc.vector.tensor_copy(out=scratch[:], in_=bias_t[:, 0:1])

            # Dequantize all of W for this core; tiles stay resident.
            w_tiles = [
                wpool.tile([P, NC], w_dt, tag="w", name=f"w{kt}") for kt in range(KT)
            ]
            for kt in range(KT):
                dt_ = dpool.tile([P, 3 * NC], f16, tag="d")
                nc.sync.dma_start(out=dt_[:], in_=d_in[kt])
                w = w_tiles[kt]
                if mode == "int16":
                    # q * s' exact in f32, then + z' and convert to int16
                    tmp = tpool.tile([P, NC], f32, tag="t")
                    nc.vector.tensor_tensor(
                        out=tmp[:], in0=dt_[:, 0:NC], in1=dt_[:, NC:2 * NC], op=mult
                    )
                    nc.vector.tensor_tensor(
                        out=w[:], in0=tmp[:], in1=dt_[:, 2 * NC:3 * NC], op=add
                    )
                else:
                    nc.vector.tensor_tensor(
                        out=w[:], in0=dt_[:, 0:NC], in1=dt_[:, NC:2 * NC], op=mult
                    )
                    nc.vector.tensor_tensor(
                        out=w[:], in0=w[:], in1=dt_[:, 2 * NC:3 * NC], op=add
                    )

            # Main GEMM: stream x m-tiles, W resident. k-outer so the 3
            # PSUM-chunk matmuls of one k share a stationary operand.
            for _rep in range(repeat):
                for mt in range(MT):
                    xt = xpool.tile([P, K], x_dt, tag="x")
                    nc.sync.dma_start(out=xt[:], in_=x_in[mt])
                    ot = opool.tile([P, NC], f16, tag="o", name=f"o{mt}")
                    pss = [
                        pspool.tile([P, PSUM_CHUNK], f32, tag="ps", name=f"ps{mt}_{i}")
                        for i in range(len(_chunks(NC)))
                    ]
                    for kt in range(KT):
                        for ci, (c0, csz) in enumerate(_chunks(NC)):
                            mm(
                                pss[ci][:, :csz],
                                xt[:, kt * P:(kt + 1) * P],
                                w_tiles[kt][:, c0:c0 + csz],
                                start=(kt == 0),
                                stop=(kt == KT - 1),
                            )
                    for ci, (c0, csz) in enumerate(_chunks(NC)):
                        if mode == "int16":
                            et = epool.tile([P, PSUM_CHUNK], f32, tag="e")
                            nc.vector.tensor_scalar(
                                out=et[:, :csz], in0=pss[ci][:, :csz],
                                scalar1=float(2.0 ** -(X_SHIFT + W_SHIFT)),
                                scalar2=None, op0=mult,
                            )
                            nc.vector.tensor_tensor(
                                out=ot[:, c0:c0 + csz], in0=et[:, :csz],
                                in1=bias_t[:, c0:c0 + csz], op=add,
                            )
                        else:
                            nc.vector.tensor_tensor(
                                out=ot[:, c0:c0 + csz], in0=pss[ci][:, :csz],
                                in1=bias_t[:, c0:c0 + csz], op=add,
                            )
                    nc.sync.dma_start(out=o_out[mt], in_=ot[:])
    nc.finalize()
    return nc


def prep_inputs(x, qweight, scales, scaled_zeros, bias, mode=MODE):
    """Host-side shard + relayout. Returns per-core in_maps."""
    B, S, K = x.shape
    N = qweight.shape[0]
    M = B * S
    NC = N // N_CORES
    MT, KT = M // P, K // P

    # x: [M, K] -> [mt, k_in, kt, m_in], replicated to every core.
    x2 = np.ascontiguousarray(
        x.reshape(MT, P, KT, P).transpose(0, 3, 2, 1)
    ).reshape(MT, P, K)
    if mode in ("bf16", "mixed"):
        x2 = x2.astype(ml_dtypes.bfloat16)
    elif mode == "int16":
        x2 = np.round(x2.astype(np.float32) * (1 << X_SHIFT)).astype(np.int16)

    qT = qweight.astype(np.float16).T  # [K, N], values 0..15 exact in fp16

    if mode == "int16":
        # power-of-two rescale keeps fp16 s'/z' exact (exponent shift only)
        sc_all = (scales.astype(np.float32) * (1 << W_SHIFT)).astype(np.float16)
        zc_all = (scaled_zeros.astype(np.float32) * (1 << W_SHIFT)).astype(np.float16)
    else:
        sc_all, zc_all = scales, scaled_zeros

    in_maps = []
    for c in range(N_CORES):
        n0 = c * NC
        dsp = np.empty((KT, P, 3 * NC), np.float16)
        dsp[:, :, 0:NC] = qT[:, n0:n0 + NC].reshape(KT, P, NC)
        dsp[:, :, NC:2 * NC] = sc_all[:, n0:n0 + NC][:, None, :]
        dsp[:, :, 2 * NC:3 * NC] = zc_all[:, n0:n0 + NC][:, None, :]
        bc = np.ascontiguousarray(np.broadcast_to(bias[n0:n0 + NC], (P, NC)))
        in_maps.append({"xp": x2, "dsp": dsp, "bp": bc})
    return in_maps


_PROG_CACHE = {}


def get_prog(M, K, NC, mode=MODE):
    key = (M, K, NC, mode)
    if key not in _PROG_CACHE:
        _PROG_CACHE[key] = build_bass(M, K, NC, mode=mode)
    return _PROG_CACHE[key]


def kernel(x, qweight, scales, scaled_zeros, bias, group_size):
    x = np.asarray(x)
    qweight = np.asarray(qweight)
    scales = np.asarray(scales)
    scaled_zeros = np.asarray(scaled_zeros)
    bias = np.asarray(bias)
    assert int(group_size) == P, f"group_size must be {P}"

    B, S, K = x.shape
    N = qweight.shape[0]
    M = B * S
    NC = N // N_CORES

    prog = get_prog(M, K, NC)
    in_maps = prep_inputs(x, qweight, scales, scaled_zeros, bias)
    res = run_bass_kernel_spmd(prog, in_maps, list(range(N_CORES))).results
    out = np.concatenate(
        [res[c]["op"].reshape(M, NC) for c in range(N_CORES)], axis=1
    )
    return out.reshape(B, S, N).astype(np.float16)


# revision 14
# speedup vs baseline: 2.3177x; 2.3177x over previous
"""AWQ fused dequant + GEMM, tensor-parallel over 8 Trainium2 NeuronCores.

Problem: out[b,s,n] = sum_k x[b,s,k] * W[n,k] + bias[n]
         W[n,k] = qweight[n,k] * scales[k//gs, n] + scaled_zeros[k//gs, n]
Shapes:  x [4,512,4096] fp16, qweight [11008,4096] int32 (values 0..15),
         scales/scaled_zeros [32,11008] fp16, bias [11008] fp16, gs=128.

Strategy (column-parallel, no collectives):
  - Shard N=11008 across 8 cores (1376 each); x replicated.
  - Host prep per core packs [q | scales' | zeros'] into one
    [KT, 128, 3*NC] fp16 tensor ("dsp"), one DMA per k-tile, so each
    on-chip dequant DVE op has exactly one DMA dependency.
  - GEMM dtype MODE (measured on HW, 512-col MMs):
      mixed: x (stationary operand) bf16, W (moving) exact fp16 —
        215 ns/MM. The PE streams at full rate only when the stationary
        dtype is bf16; a pure-fp16 matmul runs ~2x slower (504 ns/MM).
        Mixed 16-bit operand dtypes are legal and numerically correct.
        Output rel err ~3.4e-3 (x's bf16 rounding), vs 6e-4 for fp16.
      bf16 : both sides bf16 (274 ns/MM, ~4e-3 rel err).
      fp16 : exact fp16 pipeline (~2x slower PE rate, ~6e-4 rel err).
      int16: rejected by the walrus BIR verifier (kept for reference).
  - On chip per core: dequant W[k] tiles [128, NC] (resident,
    ~86KB/partition), stream x m-tiles, accumulate out[m,:] over KT
    k-matmuls per PSUM chunk (512/512/352), epilogue adds bias during
    the PSUM->SBUF copy on DVE, store [128, NC] fp16 per m-tile.
"""

import numpy as np
import ml_dtypes

import concourse.bass as bass
import concourse.mybir as mybir
from concourse import bacc
from concourse.tile import TileContext
from concourse.bass_utils import run_bass_kernel_spmd

P = 128          # partitions / PE tile
N_CORES = 8
PSUM_CHUNK = 512

MODE = "mixed"   # "fp16" | "bf16" | "mixed" | "int16"
# "mixed": x (stationary operand) in bf16, W (moving) exact fp16 — the PE
# streams at full rate when the stationary dtype is bf16, ~2x faster than
# an fp16 stationary, while W keeps fp16 precision.
X_SHIFT = 12     # x scale 2^-X_SHIFT   (int16 mode)
W_SHIFT = 15     # W scale 2^-W_SHIFT   (int16 mode)


def _chunks(n, step=PSUM_CHUNK):
    out, c0 = [], 0
    while c0 < n:
        out.append((c0, min(step, n - c0)))
        c0 += step
    return out


def _raw_matmul(nc, out, lhsT, rhs, start, stop):
    """nc.tensor.matmul without the dtype whitelist (for int16 operands)."""
    ifmap_ap = nc.tensor.lower_ap(rhs.opt({0}), opt=False)
    weights_ap = nc.tensor.lower_ap(lhsT.opt({0}), opt=False, for_matmul_weights=True)
    out_ap = nc.tensor.lower_ap(out)
    return nc.tensor.add_instruction(
        mybir.InstMatmult(
            name=nc.get_next_instruction_name(),
            replication_resolution=0,
            replication_shift_amnt=0,
            replication_num_rows=0,
            start_tensor_calc=start,
            stop_tensor_calc=stop,
            ins=[ifmap_ap, weights_ap],
            outs=[out_ap],
            perf_mode=None,
            is_transpose=None,
            ifmap_quant_offset=None,
            weights_quant_offset=None,
            bass_skip_group_check=False,
            tile_position=(0, 0),
            tile_size=(128, 128),
        )
    )


def build_bass(M, K, NC, repeat=1, mode=MODE):
    """Build the per-core SPMD Bass program.

    DRAM parameter layouts (host-prepped):
      xp  [MT, P, K]     xp[mt, k_in, kt*P + m_in] = x[mt*P+m_in, kt*P+k_in]
                         (fp16 / bf16 / int16 per mode)
      dsp [KT, P, 3*NC]  fp16, per k-tile [ qT | scales' bcast | zeros' bcast ]
      bp  [P, NC]        fp16, bias broadcast across partitions
      op  [MT, P, NC]    fp16 output, op[mt, m_in, n]
    """
    MT, KT = M // P, K // P
    f16, f32, bf16, i16 = (
        mybir.dt.float16, mybir.dt.float32, mybir.dt.bfloat16, mybir.dt.int16
    )
    mult, add = mybir.AluOpType.mult, mybir.AluOpType.add
    x_dt = {"fp16": f16, "bf16": bf16, "mixed": bf16, "int16": i16}[mode]
    w_dt = {"fp16": f16, "bf16": bf16, "mixed": f16, "int16": i16}[mode]

    nc = bacc.Bacc(None, target_bir_lowering=False, debug=True)
    x_in = nc.declare_dram_parameter("xp", [MT, P, K], x_dt, isOutput=False)
    d_in = nc.declare_dram_parameter("dsp", [KT, P, 3 * NC], f16, isOutput=False)
    b_in = nc.declare_dram_parameter("bp", [P, NC], f16, isOutput=False)
    o_out = nc.declare_dram_parameter("op", [MT, P, NC], f16, isOutput=True)

    def mm(out, lhsT, rhs, start, stop):
        if mode == "int16":
            _raw_matmul(nc, out, lhsT, rhs, start, stop)
        else:
            nc.tensor.matmul(out, lhsT, rhs, start=start, stop=stop)

    with TileContext(nc) as tc:
        with (
            tc.tile_pool(name="wpool", bufs=KT) as wpool,
            tc.tile_pool(name="xpool", bufs=3) as xpool,
            tc.tile_pool(name="dpool", bufs=3) as dpool,
            tc.tile_pool(name="tpool", bufs=2) as tpool,
            tc.tile_pool(name="cpool", bufs=1) as cpool,
            tc.tile_pool(name="opool", bufs=MT) as opool,
            tc.tile_pool(name="pspool", bufs=6, space="PSUM") as pspool,
            tc.tile_pool(name="epool", bufs=3) as epool,
        ):
            bias_t = cpool.tile([P, NC], f16)
            nc.sync.dma_start(out=bias_t[:], in_=b_in[:])
            # Make DVE observe the bias DMA's semaphore lane early so the
            # epilogue ops don't need an extra sync-wait slot for it.
            scratch = cpool.tile([P, 1], f16)
            nc.vector.tensor_copy(out=scratch[:], in_=bias_t[:, 0:1])

            # Dequantize all of W for this core; tiles stay resident.
            w_tiles = [
                wpool.tile([P, NC], w_dt, tag="w", name=f"w{kt}") for kt in range(KT)
            ]
            for kt in range(KT):
                dt_ = dpool.tile([P, 3 * NC], f16, tag="d")
                nc.sync.dma_start(out=dt_[:], in_=d_in[kt])
                w = w_tiles[kt]
                if mode == "int16":
                    # q * s' exact in f32, then + z' and convert to int16
                    tmp = tpool.tile([P, NC], f32, tag="t")
                    nc.vector.tensor_tensor(
                        out=tmp[:], in0=dt_[:, 0:NC], in1=dt_[:, NC:2 * NC], op=mult
                    )
                    nc.vector.tensor_tensor(
                        out=w[:], in0=tmp[:], in1=dt_[:, 2 * NC:3 * NC], op=add
                    )
                else:
                    nc.vector.tensor_tensor(
                        out=w[:], in0=dt_[:, 0:NC], in1=dt_[:, NC:2 * NC], op=mult
                    )
                    nc.vector.tensor_tensor(
                        out=w[:], in0=w[:], in1=dt_[:, 2 * NC:3 * NC], op=add
                    )

            # Main GEMM: stream x m-tiles, W resident. k-outer so the 3
            # PSUM-chunk matmuls of one k share a stationary operand.
            for _rep in range(repeat):
                for mt in range(MT):
                    xt = xpool.tile([P, K], x_dt, tag="x")
                    nc.sync.dma_start(out=xt[:], in_=x_in[mt])
                    ot = opool.tile([P, NC], f16, tag="o", name=f"o{mt}")
                    pss = [
                        pspool.tile([P, PSUM_CHUNK], f32, tag="ps", name=f"ps{mt}_{i}")
                        for i in range(len(_chunks(NC)))
                    ]
                    for ci, (c0, csz) in enumerate(_chunks(NC)):
                        for kt in range(KT):
                            mm(
                                pss[ci][:, :csz],
                                xt[:, kt * P:(kt + 1) * P],
                                w_tiles[kt][:, c0:c0 + csz],
                                start=(kt == 0),
                                stop=(kt == KT - 1),
                            )
                    for ci, (c0, csz) in enumerate(_chunks(NC)):
                        if mode == "int16":
                            et = epool.tile([P, PSUM_CHUNK], f32, tag="e")
                            nc.vector.tensor_scalar(
                                out=et[:, :csz], in0=pss[ci][:, :csz],
                                scalar1=float(2.0 ** -(X_SHIFT + W_SHIFT)),
                                scalar2=None, op0=mult,
                            )
                            nc.vector.tensor_tensor(
                                out=ot[:, c0:c0 + csz], in0=et[:, :csz],
                                in1=bias_t[:, c0:c0 + csz], op=add,
                            )
                        else:
                            nc.vector.tensor_tensor(
                                out=ot[:, c0:c0 + csz], in0=pss[ci][:, :csz],
                                in1=bias_t[:, c0:c0 + csz], op=add,
                            )
                    nc.sync.dma_start(out=o_out[mt], in_=ot[:])
    nc.finalize()
    return nc


def prep_inputs(x, qweight, scales, scaled_zeros, bias, mode=MODE):
    """Host-side shard + relayout. Returns per-core in_maps."""
    B, S, K = x.shape
    N = qweight.shape[0]
    M = B * S
    NC = N // N_CORES
    MT, KT = M // P, K // P

    # x: [M, K] -> [mt, k_in, kt, m_in], replicated to every core.
    x2 = np.ascontiguousarray(
        x.reshape(MT, P, KT, P).transpose(0, 3, 2, 1)
    ).reshape(MT, P, K)
    if mode in ("bf16", "mixed"):
        x2 = x2.astype(ml_dtypes.bfloat16)
    elif mode == "int16":
        x2 = np.round(x2.astype(np.float32) * (1 << X_SHIFT)).astype(np.int16)

    qT = qweight.astype(np.float16).T  # [K, N], values 0..15 exact in fp16

    if mode == "int16":
        # power-of-two rescale keeps fp16 s'/z' exact (exponent shift only)
        sc_all = (scales.astype(np.float32) * (1 << W_SHIFT)).astype(np.float16)
        zc_all = (scaled_zeros.astype(np.float32) * (1 << W_SHIFT)).astype(np.float16)
    else:
        sc_all, zc_all = scales, scaled_zeros

    in_maps = []
    for c in range(N_CORES):
        n0 = c * NC
        dsp = np.empty((KT, P, 3 * NC), np.float16)
        dsp[:, :, 0:NC] = qT[:, n0:n0 + NC].reshape(KT, P, NC)
        dsp[:, :, NC:2 * NC] = sc_all[:, n0:n0 + NC][:, None, :]
        dsp[:, :, 2 * NC:3 * NC] = zc_all[:, n0:n0 + NC][:, None, :]
        bc = np.ascontiguousarray(np.broadcast_to(bias[n0:n0 + NC], (P, NC)))
        in_maps.append({"xp": x2, "dsp": dsp, "bp": bc})
    return in_maps


_PROG_CACHE = {}


def get_prog(M, K, NC, mode=MODE):
    key = (M, K, NC, mode)
    if key not in _PROG_CACHE:
        _PROG_CACHE[key] = build_bass(M, K, NC, mode=mode)
    return _PROG_CACHE[key]


def kernel(x, qweight, scales, scaled_zeros, bias, group_size):
    x = np.asarray(x)
    qweight = np.asarray(qweight)
    scales = np.asarray(scales)
    scaled_zeros = np.asarray(scaled_zeros)
    bias = np.asarray(bias)
    assert int(group_size) == P, f"group_size must be {P}"

    B, S, K = x.shape
    N = qweight.shape[0]
    M = B * S
    NC = N // N_CORES

    prog = get_prog(M, K, NC)
    in_maps = prep_inputs(x, qweight, scales, scaled_zeros, bias)
    res = run_bass_kernel_spmd(prog, in_maps, list(range(N_CORES))).results
    out = np.concatenate(
        [res[c]["op"].reshape(M, NC) for c in range(N_CORES)], axis=1
    )
    return out.reshape(B, S, N).astype(np.float16)


# revision 19
# speedup vs baseline: 3.2232x; 1.3906x over previous
"""AWQ fused dequant + GEMM, tensor-parallel over 8 Trainium2 NeuronCores.

Problem: out[b,s,n] = sum_k x[b,s,k] * W[n,k] + bias[n]
         W[n,k] = qweight[n,k] * scales[k//gs, n] + scaled_zeros[k//gs, n]
Shapes:  x [4,512,4096] fp16, qweight [11008,4096] int32 (values 0..15),
         scales/scaled_zeros [32,11008] fp16, bias [11008] fp16, gs=128.

Strategy (column-parallel, no collectives):
  - Shard N=11008 across 8 cores (1376 each); x replicated.
  - Host prep per core packs [q | scales' | zeros'] into one
    [KT, 128, 3*NC] fp16 tensor ("dsp"), one DMA per k-tile, so each
    on-chip dequant DVE op has exactly one DMA dependency.
  - GEMM dtype MODE (measured on HW, 512-col MMs):
      mixed: x (stationary operand) bf16, W (moving) exact fp16 —
        215 ns/MM. The PE streams at full rate only when the stationary
        dtype is bf16; a pure-fp16 matmul runs ~2x slower (504 ns/MM).
        Mixed 16-bit operand dtypes are legal and numerically correct.
        Output rel err ~3.4e-3 (x's bf16 rounding), vs 6e-4 for fp16.
      bf16 : both sides bf16 (274 ns/MM, ~4e-3 rel err).
      fp16 : exact fp16 pipeline (~2x slower PE rate, ~6e-4 rel err).
      int16: rejected by the walrus BIR verifier (kept for reference).
  - On chip per core: dequant W[k] tiles [128, NC] (resident,
    ~86KB/partition), stream x m-tiles, accumulate out[m,:] over KT
    k-matmuls per PSUM chunk (512/512/352), epilogue adds bias during
    the PSUM->SBUF copy on DVE, store [128, NC] fp16 per m-tile.
"""

import numpy as np
import ml_dtypes

import concourse.bass as bass
import concourse.mybir as mybir
from concourse import bacc
from concourse.tile import TileContext
from concourse.bass_utils import run_bass_kernel_spmd

P = 128          # partitions / PE tile
N_CORES = 8
PSUM_CHUNK = 512

MODE = "mixed"   # "fp16" | "bf16" | "mixed" | "int16"
# "mixed": x (stationary operand) in bf16, W (moving) exact fp16 — the PE
# streams at full rate when the stationary dtype is bf16, ~2x faster than
# an fp16 stationary, while W keeps fp16 precision.
X_SHIFT = 12     # x scale 2^-X_SHIFT   (int16 mode)
W_SHIFT = 15     # W scale 2^-W_SHIFT   (int16 mode)


def _chunks(n, step=PSUM_CHUNK):
    out, c0 = [], 0
    while c0 < n:
        out.append((c0, min(step, n - c0)))
        c0 += step
    return out


def _raw_matmul(nc, out, lhsT, rhs, start, stop):
    """nc.tensor.matmul without the dtype whitelist (for int16 operands)."""
    ifmap_ap = nc.tensor.lower_ap(rhs.opt({0}), opt=False)
    weights_ap = nc.tensor.lower_ap(lhsT.opt({0}), opt=False, for_matmul_weights=True)
    out_ap = nc.tensor.lower_ap(out)
    return nc.tensor.add_instruction(
        mybir.InstMatmult(
            name=nc.get_next_instruction_name(),
            replication_resolution=0,
            replication_shift_amnt=0,
            replication_num_rows=0,
            start_tensor_calc=start,
            stop_tensor_calc=stop,
            ins=[ifmap_ap, weights_ap],
            outs=[out_ap],
            perf_mode=None,
            is_transpose=None,
            ifmap_quant_offset=None,
            weights_quant_offset=None,
            bass_skip_group_check=False,
            tile_position=(0, 0),
            tile_size=(128, 128),
        )
    )


def build_bass(M, K, NC, repeat=1, mode=MODE):
    """Build the per-core SPMD Bass program.

    DRAM parameter layouts (host-prepped):
      xp  [MT, P, K]     xp[mt, k_in, kt*P + m_in] = x[mt*P+m_in, kt*P+k_in]
                         (fp16 / bf16 / int16 per mode)
      dsp [KT, P, 3*NC]  fp16, per k-tile [ qT | scales' bcast | zeros' bcast ]
      bp  [P, NC]        fp16, bias broadcast across partitions
      op  [MT, P, NC]    fp16 output, op[mt, m_in, n]
    """
    MT, KT = M // P, K // P
    f16, f32, bf16, i16 = (
        mybir.dt.float16, mybir.dt.float32, mybir.dt.bfloat16, mybir.dt.int16
    )
    mult, add = mybir.AluOpType.mult, mybir.AluOpType.add
    x_dt = {"fp16": f16, "bf16": bf16, "mixed": bf16, "int16": i16}[mode]
    w_dt = {"fp16": f16, "bf16": bf16, "mixed": f16, "int16": i16}[mode]

    nc = bacc.Bacc(None, target_bir_lowering=False, debug=True)
    x_in = nc.declare_dram_parameter("xp", [MT, P, K], x_dt, isOutput=False)
    d_in = nc.declare_dram_parameter("dsp", [KT, P, 3 * NC], f16, isOutput=False)
    b_in = nc.declare_dram_parameter("bp", [P, NC], f16, isOutput=False)
    o_out = nc.declare_dram_parameter("op", [MT, P, NC], f16, isOutput=True)

    def mm(out, lhsT, rhs, start, stop):
        if mode == "int16":
            _raw_matmul(nc, out, lhsT, rhs, start, stop)
        else:
            nc.tensor.matmul(out, lhsT, rhs, start=start, stop=stop)

    with TileContext(nc) as tc:
        with (
            tc.tile_pool(name="wpool", bufs=KT) as wpool,
            tc.tile_pool(name="xpool", bufs=4) as xpool,
            tc.tile_pool(name="dpool", bufs=3) as dpool,
            tc.tile_pool(name="tpool", bufs=2) as tpool,
            tc.tile_pool(name="cpool", bufs=1) as cpool,
            tc.tile_pool(name="opool", bufs=MT) as opool,
            tc.tile_pool(name="pspool", bufs=6, space="PSUM") as pspool,
            tc.tile_pool(name="epool", bufs=3) as epool,
        ):
            bias_t = cpool.tile([P, NC], f16)
            nc.sync.dma_start(out=bias_t[:], in_=b_in[:])
            # Make DVE observe the bias DMA's semaphore lane early so the
            # epilogue ops don't need an extra sync-wait slot for it.
            scratch = cpool.tile([P, 1], f16)
            nc.vector.tensor_copy(out=scratch[:], in_=bias_t[:, 0:1])

            # Prefetch the first x m-tiles on the ACT HWDGE ring
            # (nc.scalar) so they are not FIFO-queued behind the 33MB of
            # dsp weight DMAs on the SP ring — otherwise the PE's first
            # matmul stalls until the whole weight stream has drained.
            x_prefetch = {}
            for mt in range(min(2, MT)):
                xt = xpool.tile([P, K], x_dt, tag="x", name=f"xpre{mt}")
                nc.scalar.dma_start(out=xt[:], in_=x_in[mt])
                x_prefetch[mt] = xt

            # Dequantize all of W for this core; tiles stay resident.
            w_tiles = [
                wpool.tile([P, NC], w_dt, tag="w", name=f"w{kt}") for kt in range(KT)
            ]
            for kt in range(KT):
                dt_ = dpool.tile([P, 3 * NC], f16, tag="d")
                nc.sync.dma_start(out=dt_[:], in_=d_in[kt])
                w = w_tiles[kt]
                if mode == "int16":
                    # q * s' exact in f32, then + z' and convert to int16
                    tmp = tpool.tile([P, NC], f32, tag="t")
                    nc.vector.tensor_tensor(
                        out=tmp[:], in0=dt_[:, 0:NC], in1=dt_[:, NC:2 * NC], op=mult
                    )
                    nc.vector.tensor_tensor(
                        out=w[:], in0=tmp[:], in1=dt_[:, 2 * NC:3 * NC], op=add
                    )
                else:
                    nc.vector.tensor_tensor(
                        out=w[:], in0=dt_[:, 0:NC], in1=dt_[:, NC:2 * NC], op=mult
                    )
                    nc.vector.tensor_tensor(
                        out=w[:], in0=w[:], in1=dt_[:, 2 * NC:3 * NC], op=add
                    )

            # Main GEMM: stream x m-tiles, W resident. k-outer so the 3
            # PSUM-chunk matmuls of one k share a stationary operand.
            for _rep in range(repeat):
                for mt in range(MT):
                    xt = x_prefetch.pop(mt, None) if _rep == 0 else None
                    if xt is None:
                        xt = xpool.tile([P, K], x_dt, tag="x")
                        nc.scalar.dma_start(out=xt[:], in_=x_in[mt])
                    ot = opool.tile([P, NC], f16, tag="o", name=f"o{mt}")
                    pss = [
                        pspool.tile([P, PSUM_CHUNK], f32, tag="ps", name=f"ps{mt}_{i}")
                        for i in range(len(_chunks(NC)))
                    ]
                    for ci, (c0, csz) in enumerate(_chunks(NC)):
                        for kt in range(KT):
                            mm(
                                pss[ci][:, :csz],
                                xt[:, kt * P:(kt + 1) * P],
                                w_tiles[kt][:, c0:c0 + csz],
                                start=(kt == 0),
                                stop=(kt == KT - 1),
                            )
                    for ci, (c0, csz) in enumerate(_chunks(NC)):
                        if mode == "int16":
                            et = epool.tile([P, PSUM_CHUNK], f32, tag="e")
                            nc.vector.tensor_scalar(
                                out=et[:, :csz], in0=pss[ci][:, :csz],
                                scalar1=float(2.0 ** -(X_SHIFT + W_SHIFT)),
                                scalar2=None, op0=mult,
                            )
                            nc.vector.tensor_tensor(
                                out=ot[:, c0:c0 + csz], in0=et[:, :csz],
                                in1=bias_t[:, c0:c0 + csz], op=add,
                            )
                        else:
                            nc.vector.tensor_tensor(
                                out=ot[:, c0:c0 + csz], in0=pss[ci][:, :csz],
                                in1=bias_t[:, c0:c0 + csz], op=add,
                            )
                    nc.sync.dma_start(out=o_out[mt], in_=ot[:])
    nc.finalize()
    return nc


def prep_inputs(x, qweight, scales, scaled_zeros, bias, mode=MODE):
    """Host-side shard + relayout. Returns per-core in_maps."""
    B, S, K = x.shape
    N = qweight.shape[0]
    M = B * S
    NC = N // N_CORES
    MT, KT = M // P, K // P

    # x: [M, K] -> [mt, k_in, kt, m_in], replicated to every core.
    x2 = np.ascontiguousarray(
        x.reshape(MT, P, KT, P).transpose(0, 3, 2, 1)
    ).reshape(MT, P, K)
    if mode in ("bf16", "mixed"):
        x2 = x2.astype(ml_dtypes.bfloat16)
    elif mode == "int16":
        x2 = np.round(x2.astype(np.float32) * (1 << X_SHIFT)).astype(np.int16)

    qT = qweight.astype(np.float16).T  # [K, N], values 0..15 exact in fp16

    if mode == "int16":
        # power-of-two rescale keeps fp16 s'/z' exact (exponent shift only)
        sc_all = (scales.astype(np.float32) * (1 << W_SHIFT)).astype(np.float16)
        zc_all = (scaled_zeros.astype(np.float32) * (1 << W_SHIFT)).astype(np.float16)
    else:
        sc_all, zc_all = scales, scaled_zeros

    in_maps = []
    for c in range(N_CORES):
        n0 = c * NC
        dsp = np.empty((KT, P, 3 * NC), np.float16)
        dsp[:, :, 0:NC] = qT[:, n0:n0 + NC].reshape(KT, P, NC)
        dsp[:, :, NC:2 * NC] = sc_all[:, n0:n0 + NC][:, None, :]
        dsp[:, :, 2 * NC:3 * NC] = zc_all[:, n0:n0 + NC][:, None, :]
        bc = np.ascontiguousarray(np.broadcast_to(bias[n0:n0 + NC], (P, NC)))
        in_maps.append({"xp": x2, "dsp": dsp, "bp": bc})
    return in_maps


_PROG_CACHE = {}


def get_prog(M, K, NC, mode=MODE):
    key = (M, K, NC, mode)
    if key not in _PROG_CACHE:
        _PROG_CACHE[key] = build_bass(M, K, NC, mode=mode)
    return _PROG_CACHE[key]


def kernel(x, qweight, scales, scaled_zeros, bias, group_size):
    x = np.asarray(x)
    qweight = np.asarray(qweight)
    scales = np.asarray(scales)
    scaled_zeros = np.asarray(scaled_zeros)
    bias = np.asarray(bias)
    assert int(group_size) == P, f"group_size must be {P}"

    B, S, K = x.shape
    N = qweight.shape[0]
    M = B * S
    NC = N // N_CORES

    prog = get_prog(M, K, NC)
    in_maps = prep_inputs(x, qweight, scales, scaled_zeros, bias)
    res = run_bass_kernel_spmd(prog, in_maps, list(range(N_CORES))).results
    out = np.concatenate(
        [res[c]["op"].reshape(M, NC) for c in range(N_CORES)], axis=1
    )
    return out.reshape(B, S, N).astype(np.float16)
